# revision 16
# baseline (speedup 1.0000x reference)
"""Trainium2 Bass kernel for nn_AdjacencyMatrix (gnn_message_passing).

Math: the reference keeps state = W * v[:, None] at all times, where
  v0 = pad(x[0], n),  v_{t+1} = W^T v_t  (colsum of state),
and the output is diag(W)[-256:] * v_num_steps[-256:].

So the whole problem collapses to num_steps matvecs v <- W^T v plus an
elementwise multiply by the last 256 diagonal entries of W.  Step 1 only
needs rows 0:1024 of W (v0 is zero elsewhere); the last step only needs
the last 256 columns of W.

Sharding (8 cores): row-parallel. Core d owns rows r_d = [1024d, 1024d+1024).
 - v1[r_d] = W[0:1024, r_d]^T x                      (local, no collective)
 - middle steps: partial = W[r_d,:]^T v[r_d] -> AllToAll + local 8-way sum
   gives core d exactly v_next[r_d]  (A2A is far cheaper than RS/AR on 8
   LNC1 ranks: mesh-style vs a 14-step ring)
 - final step: partial256 = W[r_d, -256:]^T v[r_d] -> AllReduce
 - out = diag * v_last[-256:]                        (identical on all cores)

W is stored/streamed as bf16 (vector math stays fp32): halves HBM traffic
and the whole 16 MiB row-shard stays SBUF-resident, so the second middle
pass re-reads nothing.

Layout convention: per-core vectors live in SBUF as [128, 8] tiles with
(p, k) = v[1024d + 8p + k]; W k-tile k holds rows {8p + k} (strided DMA),
so collective results load/sum directly into that layout with zero
transposes.
"""

import ml_dtypes
import numpy as np

import concourse.bass as bass
import concourse.mybir as mybir
from concourse import bacc, tile
from concourse.bass_utils import run_bass_kernel_spmd

N = 8192
SEG = 256 // 8            # 32 output elements per core
IN_N = 1024
OUT_N = 256
NCORES = 8
RP = N // NCORES          # rows per core = 1024
KT = RP // 128            # k-tiles per core = 8
D0 = N - OUT_N            # 7936

F32 = mybir.dt.float32
BF16 = mybir.dt.bfloat16
RG = [list(range(NCORES))]

PANEL = 2048              # W columns per DMA panel (compute granularity)
CH = 512                  # psum chunk (one fp32 PSUM bank)

_cache: dict = {}


def _build(num_steps: int):
    """Build + compile the SPMD graph for num_steps >= 2."""
    n_mid = num_steps - 2
    nc = bacc.Bacc(
        "TRN2", target_bir_lowering=False, debug=False, num_devices=NCORES
    )
    xT = nc.declare_dram_parameter("xT", [128, 8], BF16, isOutput=False)
    A = nc.declare_dram_parameter("A", [IN_N, RP], BF16, isOutput=False)
    Wr = nc.declare_dram_parameter("Wr", [RP, N], BF16, isOutput=False)
    Wc = nc.declare_dram_parameter("Wc", [RP, OUT_N], BF16, isOutput=False)
    dg = nc.declare_dram_parameter("dg", [1, SEG], F32, isOutput=False)
    out = nc.declare_dram_parameter("out", [1, SEG], F32, isOutput=True)

    Wr_il = Wr.ap().rearrange("(p e) c -> e p c", e=KT)   # [8,128,8192] row 8p+k
    Wc_il = Wc.ap().rearrange("(p e) c -> e p c", e=KT)

    with tile.TileContext(nc) as tc:
        with (
            tc.tile_pool(name="small", bufs=1) as small,
            tc.tile_pool(name="apool", bufs=1) as apool,
            tc.tile_pool(name="wres", bufs=1) as wres,
            tc.tile_pool(name="ppool", bufs=4, space="PSUM") as ppool,
            tc.tile_pool(name="pp1", bufs=1, space="PSUM") as pp1,
            tc.tile_pool(name="dram", bufs=1, space="DRAM") as dram,
        ):
            # ---------------- stage 1: u1 = A^T x (local v1 slice) ----------
            xt = small.tile([128, 8], BF16, name="xt")
            nc.gpsimd.dma_start(out=xt[:, :], in_=xT.ap())
            a_sb = apool.tile([128, KT * RP], BF16, name="a_sb")
            for k in range(KT):
                nc.sync.dma_start(
                    out=a_sb[:, k * RP:(k + 1) * RP],
                    in_=A.ap()[k * 128:(k + 1) * 128, :],
                )
            u1_ps = pp1.tile([128, 8], F32, name="u1_ps")
            for m in range(8):
                for k in range(KT):
                    nc.tensor.matmul(
                        u1_ps[:, m:m + 1],
                        lhsT=a_sb[:, k * RP + m * 128: k * RP + (m + 1) * 128],
                        rhs=xt[:, k:k + 1],
                        start=(k == 0),
                        stop=(k == KT - 1),
                    )
            u_cur = small.tile([128, 8], BF16, name="u1")
            nc.vector.tensor_copy(u_cur[:, :], u1_ps[:, :])

            partial = small.tile([1, N], F32, name="partial")
            ones8 = small.tile([8, 1], F32, name="ones8")
            nc.vector.memset(ones8[0:8, :], 1.0)

            # resident W row-shard: 8 k-tiles of [128, 8192] bf16 (16 MiB)
            wk = [
                wres.tile([128, N], BF16, name=f"wk_{k}") for k in range(KT)
            ]
            # early small prefetches for the final stage
            wc = small.tile([128, KT * OUT_N], BF16, name="wc")
            for k in range(KT):
                nc.scalar.dma_start(
                    out=wc[:, k * OUT_N:(k + 1) * OUT_N], in_=Wc_il[k]
                )
            dgt = small.tile([1, SEG], F32, name="dgt")
            nc.scalar.dma_start(out=dgt[0:1, :], in_=dg.ap())

            # ---------------- middle steps (num_steps - 2 of them) ----------
            for s in range(n_mid):
                for j in range(N // PANEL):
                    if s == 0:
                        # panel-major DMA into the resident k-tile tiles
                        for k in range(KT):
                            nc.sync.dma_start(
                                out=wk[k][:, j * PANEL:(j + 1) * PANEL],
                                in_=Wr_il[k][:, j * PANEL:(j + 1) * PANEL],
                            )
                    for c in range(PANEL // CH):
                        col = j * PANEL + c * CH
                        ps = ppool.tile(
                            [1, CH], F32, name=f"ps_{s}_{j}_{c}", tag="ps"
                        )
                        for k in range(KT):
                            nc.tensor.matmul(
                                ps[0:1, :],
                                lhsT=u_cur[:, k:k + 1],
                                rhs=wk[k][:, col:col + CH],
                                start=(k == 0),
                                stop=(k == KT - 1),
                            )
                        nc.scalar.copy(
                            out=partial[0:1, col:col + CH],
                            in_=ps[0:1, :],
                        )
                # AllToAll (slot j of input -> core j) + local 8-way sum.
                cc_in = dram.tile([1, N], F32, name=f"cc_in_{s}")
                cc_out = dram.tile([NCORES, RP], F32, name=f"cc_out_{s}")
                nc.gpsimd.dma_start(out=cc_in[:, :], in_=partial[0:1, :])
                nc.gpsimd.collective_compute(
                    "AllToAll",
                    mybir.AluOpType.bypass,
                    replica_groups=RG,
                    ins=[cc_in.opt()],
                    outs=[cc_out.opt()],
                )
                acc = small.tile([NCORES, RP], F32, name=f"acc_{s}", tag="acc")
                nc.gpsimd.dma_start(out=acc[0:NCORES, :], in_=cc_out[:, :])
                acc3 = acc[0:NCORES, :].rearrange("s (p k) -> k s p", k=8)
                un_ps = ppool.tile(
                    [128, 8], F32, name=f"un_ps_{s}", tag="unps", bufs=1
                )
                for k in range(8):
                    nc.tensor.matmul(
                        un_ps[:, k:k + 1],
                        lhsT=acc3[k],
                        rhs=ones8[0:NCORES, 0:1],
                        start=True,
                        stop=True,
                    )
                u_next = small.tile([128, 8], BF16, name=f"u_{s + 2}")
                nc.vector.tensor_copy(u_next[:, :], un_ps[:, :])
                u_cur = u_next

            # ---------------- final step: last 256 columns ------------------
            ps4 = pp1.tile([1, OUT_N], F32, name="ps4")
            for k in range(KT):
                nc.tensor.matmul(
                    ps4[0:1, :],
                    lhsT=u_cur[:, k:k + 1],
                    rhs=wc[:, k * OUT_N:(k + 1) * OUT_N],
                    start=(k == 0),
                    stop=(k == KT - 1),
                )
            p4 = small.tile([1, OUT_N], F32, name="p4")
            nc.vector.tensor_copy(p4[0:1, :], ps4[0:1, :])
            cc4_in = dram.tile([1, OUT_N], F32, name="cc4_in")
            cc4_out = dram.tile([NCORES, SEG], F32, name="cc4_out")
            nc.gpsimd.dma_start(out=cc4_in[:, :], in_=p4[0:1, :])
            nc.gpsimd.collective_compute(
                "AllToAll",
                mybir.AluOpType.bypass,
                replica_groups=RG,
                ins=[cc4_in.opt()],
                outs=[cc4_out.opt()],
            )
            acc4 = small.tile([NCORES, SEG], F32, name="acc4")
            nc.gpsimd.dma_start(out=acc4[0:NCORES, :], in_=cc4_out[:, :])
            v4_ps = pp1.tile([1, SEG], F32, name="v4_ps")
            nc.tensor.matmul(
                v4_ps[0:1, :],
                lhsT=ones8[0:NCORES, 0:1],
                rhs=acc4[0:NCORES, :],
                start=True,
                stop=True,
            )
            v4 = small.tile([1, SEG], F32, name="v4")
            nc.vector.tensor_copy(v4[0:1, :], v4_ps[0:1, :])
            res = small.tile([1, SEG], F32, name="res")
            nc.vector.tensor_mul(res[0:1, :], v4[0:1, :], dgt[0:1, :])
            nc.gpsimd.dma_start(out=out.ap(), in_=res[0:1, :])

    nc.compile()
    return nc


def _get(num_steps: int):
    if num_steps not in _cache:
        _cache[num_steps] = _build(num_steps)
    return _cache[num_steps]


def _shard_inputs(x: np.ndarray, W: np.ndarray):
    bf = ml_dtypes.bfloat16
    xT = np.ascontiguousarray(x[0].reshape(8, 128).T).astype(bf)
    dgv = np.ascontiguousarray(np.diagonal(W)[D0:]).astype(np.float32)
    in_maps = []
    for d in range(NCORES):
        blk = W[0:IN_N, RP * d: RP * (d + 1)]
        # column c of the device A must be W_block[:, 8p+m] for c = m*128+p
        A = np.ascontiguousarray(
            blk.reshape(IN_N, 128, 8).transpose(0, 2, 1).reshape(IN_N, RP)
        ).astype(bf)
        Wr = np.ascontiguousarray(W[RP * d: RP * (d + 1), :]).astype(bf)
        Wc = np.ascontiguousarray(Wr[:, D0:])
        dg_d = np.ascontiguousarray(dgv[SEG * d: SEG * (d + 1)]).reshape(1, SEG)
        in_maps.append({"xT": xT, "A": A, "Wr": Wr, "Wc": Wc, "dg": dg_d})
    return in_maps


def _run(x, W, num_steps, trace=False):
    x = np.asarray(x, dtype=np.float32)
    W = np.asarray(W, dtype=np.float32)
    num_steps = int(num_steps)
    if num_steps == 0:
        # v0 is zero on the last 256 entries (x only fills the first 1024)
        return np.zeros(OUT_N, np.float32), None
    if num_steps == 1:
        # out = diag * v1[-256:]; tiny, never hit by the harness (4 steps)
        v1d = W[0:IN_N, D0:].T.astype(np.float64) @ x[0].astype(np.float64)
        return (np.diagonal(W)[D0:] * v1d).astype(np.float32), None
    nc = _get(num_steps)
    in_maps = _shard_inputs(x, W)
    r = run_bass_kernel_spmd(
        nc, in_maps, core_ids=list(range(NCORES)), trace=trace
    )
    outv = np.concatenate(
        [np.asarray(r.results[d]["out"], np.float32).reshape(SEG)
         for d in range(NCORES)]
    )
    return outv, r


def kernel(x, W, num_steps) -> np.ndarray:
    outv, _ = _run(x, W, num_steps, trace=False)
    return outv


def run_traced(x, W, num_steps):
    return _run(x, W, num_steps, trace=True)
